# revision 87
# baseline (speedup 1.0000x reference)
"""GQA kernel for Trainium2, 8 NeuronCores (all-bf16, fused pipeline).

Sharding: core c -> batch b = c//4, kv-head-group g = c%4.
Each core handles 1 batch, 2 KV heads (2g, 2g+1), 8 Q heads (8g..8g+7),
row-shard of W_o (rows 512g..512g+512). Host sums the 4 partial outputs
per batch and adds bo.

All matmul operands are bf16 (1 PE cycle/row at any moving size; fp32
PSUM accumulation), validated to ~4e-3 rel err vs the fp32 reference.

The kernel is ONE fused instruction stream: projections for t-group tg
are interleaved between attention head-blocks so PE always has
independent work while the Activation engine drains exponentials
(attention q-group qg only needs projections up to tg=2qg+1).

  1. QKV projections from x^T (E on partitions), weights DMA'd directly
     as bf16.  Q scaled by 1/8 on host.  V is produced directly in
     [t, d] layout by making x^T the stationary operand, with a ones
     column per key block so the attention matmul also yields the
     softmax denominator.
  2. Attention per (qg, qh), both kv chains kb-interleaved in key-block
     pairs: S^T = K^T.T Q^T per 128-wide key block (two blocks share one
     PSUM tile and one exp), causality via block skip + a post-exp 0/1
     triangular mask multiply on Pool (off the scores->exp chain), exp
     on ACT (no max subtraction: |scores| <= ~7), A V computed as
     out[q, 0:65] = at_block.T @ [V|1] (65-row moving operand).  The AV
     accumulator uses a single PSUM start (start=True zeroes the whole
     2KB zero region; later q-block regions rely on its pending-zero).
  3. Normalize rows by 1/l via DVE reciprocal + tensor_scalar multiply
     into a [q, d_kv0|d_kv1] tile, DMA-XBAR-transposed into attn[hh]
     as [d, q] (GPSIMD cannot touch PSUM; DVE drains all PSUM reads).
  4. O-projection partial = attn^T.T @ Wo_shard, drip-fed as 256-col
     units into the attention stream (paced per q-group), PSUM drained
     on DVE/ACT, streamed to DRAM in 1024-col halves.
"""

import numpy as np
import ml_dtypes

BF = ml_dtypes.bfloat16

E = 2048
S = 2048
B = 2
D = 64
NCORE = 8
TGW = 256          # t-column group width in projections
NTG = S // TGW     # 8
EC = E // 128      # 16 contraction chunks
NKB = S // 128     # 16 key blocks
NQG = S // 512     # 4 q column groups

_CACHE = {}
# tile jb holds q-heads (jb, jb+4): kv0 heads at base partition 0,
# kv1 heads at base partition 64, matching the K/V partition layout
HEAD_PERM = [0, 4, 1, 5, 2, 6, 3, 7]


def _build():
    import concourse.bass as bass
    import concourse.tile as tile
    from concourse import mybir, bacc

    F32 = mybir.dt.float32
    BF16 = mybir.dt.bfloat16
    Exp = mybir.ActivationFunctionType.Exp


    nc = bacc.Bacc("TRN2", target_bir_lowering=False, debug=False,
                   num_devices=NCORE)

    XT = nc.declare_dram_parameter("xt", [128, EC, S], BF16, isOutput=False)
    WQ = nc.declare_dram_parameter("wq", [128, 4, EC, 128], BF16,
                                   isOutput=False)
    WK = nc.declare_dram_parameter("wk", [128, EC, 128], BF16, isOutput=False)
    WV = nc.declare_dram_parameter("wv", [128, EC, 128], BF16, isOutput=False)
    WO = nc.declare_dram_parameter("wo", [128, 4, E], BF16, isOutput=False)
    BIAS = nc.declare_dram_parameter("bias", [128, 6], F32, isOutput=False)
    BIASV = nc.declare_dram_parameter("biasv", [128, 128], F32, isOutput=False)
    CM = nc.declare_dram_parameter("cmask", [128, 128], BF16, isOutput=False)
    # output streamed as bf16: halves the 16.8MB of output DMA (the
    # fp32->bf16 cast rides the existing PSUM-drain copies; host upcasts)
    OUT = nc.declare_dram_parameter("out", [S, E], BF16, isOutput=True)

    with tile.TileContext(nc) as tc:
        with tc.tile_pool(name="persist", bufs=1) as persist, \
             tc.tile_pool(name="xr", bufs=3) as xrp, \
             tc.tile_pool(name="at", bufs=16) as atp, \
             tc.tile_pool(name="nrm", bufs=10) as npool, \
             tc.tile_pool(name="rcp", bufs=8) as rpool, \
             tc.tile_pool(name="ostage", bufs=3) as osp, \
             tc.tile_pool(name="st", bufs=2, space="PSUM") as stp, \
             tc.tile_pool(name="av", bufs=2, space="PSUM") as avp, \
             tc.tile_pool(name="acc", bufs=2, space="PSUM") as accp:

            qt = [persist.tile([128, S], BF16, tag=f"qt{i}", name=f"qt{i}")
                  for i in range(4)]
            kt = persist.tile([128, S], BF16, tag="kt")
            # per key block kb: cols 0:64 = V_kv0, 64 = ones, 65:129 = V_kv1,
            # 129 = ones  (ones give the softmax denominator in the AV matmul)
            vt = persist.tile([128, NKB, 130], BF16, tag="vt")
            attn = [persist.tile([128, S], BF16, tag=f"attn{i}",
                                 name=f"attn{i}") for i in range(4)]
            wq_r = persist.tile([128, 4, EC, 128], BF16, tag="wq")
            wk_r = persist.tile([128, EC, 128], BF16, tag="wk")
            wv_r = persist.tile([128, EC, 128], BF16, tag="wv")
            wo_r = persist.tile([128, 4, E], BF16, tag="wo")
            # 0/1 upper-triangular mask, applied post-exp (keeps the Pool
            # engine off the scores->exp critical chain)
            cm = persist.tile([128, 128], BF16, tag="cm")
            bias_t = persist.tile([128, 6], F32, tag="bias")
            biasv_t = persist.tile([128, 128], F32, tag="biasv")

            # weight/bias DMAs, ordered so the first Q-proj chain can start
            # as early as possible (DMA engines are a serial resource)
            nc.sync.dma_start(out=wq_r[:, 0], in_=WQ[:, 0])
            xr0 = xrp.tile([128, EC, TGW], BF16, tag="xr", name="xr0")
            nc.sync.dma_start(out=xr0, in_=XT[:, :, 0:TGW])
            nc.sync.dma_start(out=wq_r[:, 1], in_=WQ[:, 1])
            nc.sync.dma_start(out=wq_r[:, 2], in_=WQ[:, 2])
            nc.sync.dma_start(out=wk_r, in_=WK[:, :, :])
            nc.sync.dma_start(out=wq_r[:, 3], in_=WQ[:, 3])
            nc.sync.dma_start(out=wv_r, in_=WV[:, :, :])
            nc.sync.dma_start(out=bias_t, in_=BIAS[:, :])
            nc.sync.dma_start(out=biasv_t, in_=BIASV[:, :])
            nc.sync.dma_start(out=cm, in_=CM[:, :])

            nc.gpsimd.memset(vt[:, :, 64:65], 1.0)
            nc.gpsimd.memset(vt[:, :, 129:130], 1.0)

            # ---- drip-fed filler units (projection chains + O-proj
            # chunks) interleaved into the attention kb loop: PE always has
            # independent ~0.5-1.7us work while ACT drains exponentials.
            xrs = {0: xr0}

            def xslice(tg, ec):
                return xrs[tg][:, ec, :]
            proj_q = []
            pending = []
            stage = {}

            def ensure_xr(tg):
                if tg in xrs or tg >= NTG:
                    return
                xr = xrp.tile([128, EC, TGW], BF16, tag="xr", name="xr")
                nc.sync.dma_start(out=xr,
                                  in_=XT[:, :, tg * TGW:(tg + 1) * TGW])
                xrs[tg] = xr

            def queue_proj(tg):
                proj_q.extend((tg, k) for k in range(7))

            def do_proj_unit():
                tg, k = proj_q.pop(0)
                if k == 0:
                    ensure_xr(tg + 1)
                cols = bass.ds(tg * TGW, TGW)
                if k < 5:
                    ps = accp.tile([128, TGW], F32, tag="acc", name="ps")
                    for ec in range(EC):
                        lhsT = wq_r[:, k, ec, :] if k < 4 else wk_r[:, ec, :]
                        nc.tensor.matmul(ps, lhsT, xslice(tg, ec),
                                         start=(ec == 0), stop=(ec == EC - 1))
                    if k < 4:
                        nc.vector.tensor_scalar_add(qt[k][:, cols], ps,
                                                    bias_t[:, k:k + 1])
                    else:
                        nc.vector.tensor_scalar_add(kt[:, cols], ps,
                                                    bias_t[:, 4:5])
                else:
                    # V directly in [t, d] layout: x^T chunk is stationary
                    h2 = k - 5
                    tb = 2 * tg + h2
                    psv = accp.tile([128, 128], F32, tag="acc", name="psv")
                    for ec in range(EC):
                        nc.tensor.matmul(
                            psv,
                            xslice(tg, ec)[:, h2 * 128:(h2 + 1) * 128],
                            wv_r[:, ec, :],
                            start=(ec == 0), stop=(ec == EC - 1))
                    nc.vector.tensor_add(vt[:, tb, 0:64], psv[:, 0:64],
                                         biasv_t[:, 0:64])
                    nc.vector.tensor_add(vt[:, tb, 65:129], psv[:, 64:128],
                                         biasv_t[:, 64:128])
                if tg == 1 and k == 6:
                    nc.sync.dma_start(out=wo_r, in_=WO[:, :, :])

            def queue_oproj(tb):
                pending.extend(('full', tb, ng) for ng in range(8))

            cur_qg = [0]
            flip = [0]

            def do_oproj_unit():
                kind, tb, ng = pending.pop(0)
                if ng == 0:
                    stage[tb] = osp.tile([128, E], BF16, tag="ostage",
                                         name="ostage")
                op = accp.tile([128, 256], F32, tag="acc", name="op")
                for jc in range(4):
                    nc.tensor.matmul(
                        op, attn[jc][:, tb * 128:(tb + 1) * 128],
                        wo_r[:, jc, ng * 256:(ng + 1) * 256],
                        start=(jc == 0), stop=(jc == 3),
                        skip_group_check=True)
                flip[0] ^= 1
                # ACT has slack until the attention stream deepens (qg>=2)
                if cur_qg[0] < 2 and flip[0]:
                    nc.scalar.activation(
                        stage[tb][:, ng * 256:(ng + 1) * 256], op,
                        mybir.ActivationFunctionType.Copy)
                else:
                    nc.vector.tensor_copy(
                        stage[tb][:, ng * 256:(ng + 1) * 256], op)
                if ng == 3 or ng == 7:
                    h0 = (ng - 3) * 256
                    nc.sync.dma_start(
                        out=OUT[tb * 128:(tb + 1) * 128, h0:h0 + 1024],
                        in_=stage[tb][:, h0:h0 + 1024])
                    if ng == 7:
                        stage.pop(tb)

            # pacing: drain the filler queues evenly over a qg's kb-substeps
            pace = {'n': 1, 'i': 0, 'tot': 0, 'done': 0}

            def fill_tick():
                pace['i'] += 1
                target = pace['tot'] * pace['i'] // pace['n']
                while pace['done'] < target and (proj_q or pending):
                    if proj_q:
                        do_proj_unit()
                    else:
                        do_oproj_unit()
                    pace['done'] += 1

            def attn_hh(qg, hh):
                q0 = qg * 512
                nkb = qg * 4 + 4
                # nrm2[qb] = normalized attn rows [q, d(kv0)|d(kv1)],
                # DMA-XBAR-transposed into attn[hh] as [d, q]
                nrm2 = [npool.tile([128, 128], BF16, tag="nrm",
                                   name=f"nrm{qb}") for qb in range(4)]
                # av[kv][:, qb, 0:65] accumulates [q, V|l] per head
                av = [avp.tile([128, 4, 128], F32, tag="av", name="av")
                      for _ in range(2)]
                pend = [[], []]

                def flush(kv, pair, at):
                    for kb, c0, off in pair:
                        for qb in range((c0 - q0) // 128, 4):
                            col = off + qb * 128 - (c0 - q0)
                            # start only on the tile's very first matmul:
                            # start=True zeroes the whole 2KB PSUM zero
                            # region, so later qb regions must rely on the
                            # pending-zero from that first start
                            nc.tensor.matmul(
                                av[kv][:, qb, 0:65],
                                at[:, col:col + 128],
                                vt[:, kb, kv * 65:kv * 65 + 65],
                                start=(kb == 0 and qb == 0),
                                stop=(kb == qg * 4 + qb),
                                skip_group_check=True)

                for p in range(nkb // 2):
                    for kv in range(2):
                        qoff = kv * 64
                        st = stp.tile([128, 1024], F32, tag="st", name="st")
                        pair = []
                        off = 0
                        for kb in (2 * p, 2 * p + 1):
                            c0 = max(kb * 128, q0)
                            w = q0 + 512 - c0
                            nc.tensor.matmul(
                                st[:, off:off + w],
                                kt[qoff:qoff + 64, kb * 128:(kb + 1) * 128],
                                qt[hh][qoff:qoff + 64, c0:c0 + w],
                                start=True, stop=True, skip_group_check=True)
                            pair.append((kb, c0, off))
                            off += w
                        at = atp.tile([128, 1024], BF16, tag="at", name="at")
                        nc.scalar.activation(at[:, :off], st[:, :off], Exp)
                        for kb, c0, koff in pair:
                            if kb * 128 >= q0:
                                nc.gpsimd.tensor_mul(at[:, koff:koff + 128],
                                                     at[:, koff:koff + 128],
                                                     cm)
                        pend[kv].append((pair, at))
                        if len(pend[kv]) > 7:
                            flush(kv, *pend[kv].pop(0))
                        fill_tick()
                for kv in range(2):
                    while pend[kv]:
                        flush(kv, *pend[kv].pop(0))
                    for qb in range(4):
                        rcp = rpool.tile([128, 1], F32, tag="rcp",
                                         name="rcp")
                        nc.vector.reciprocal(rcp, av[kv][:, qb, 64:65])
                        nc.vector.tensor_scalar_mul(
                            nrm2[qb][:, kv * 64:kv * 64 + 64],
                            av[kv][:, qb, 0:64], rcp)
                for qb in range(4):
                    qcol = q0 + qb * 128
                    nc.sync.dma_start_transpose(
                        out=attn[hh][:, qcol:qcol + 128], in_=nrm2[qb])

            # ---- fused schedule: projections feed attention just-in-time
            # (tg0/tg1 Q and K chains run up front; their V chains are only
            # needed by the first AV flush, so they drip in as early filler)
            proj_q.extend((tg, k) for tg in (0, 1) for k in range(5))
            for _ in range(10):
                do_proj_unit()
            proj_q.extend((tg, k) for tg in (0, 1) for k in (5, 6))
            for tg in range(2, NTG):
                queue_proj(tg)
            for qg in range(NQG):
                cur_qg[0] = qg
                for tb in range(max(qg - 1, 0) * 4, qg * 4):
                    queue_oproj(tb)
                nslots = 4 * (qg * 4 + 4)
                navail = len(proj_q) + len(pending)
                # early qgs: up to 1 unit per substep (surplus spills to the
                # next qg); qg3: spread the remaining units over all substeps
                tot = navail if qg == 3 else nslots
                if qg == 2:
                    tot = nslots - 32
                elif qg == 1:
                    tot = nslots - 16
                pace.update(n=nslots, i=0, done=0, tot=tot)
                for hh in range(4):
                    attn_hh(qg, hh)
            while proj_q or pending:
                if proj_q:
                    do_proj_unit()
                else:
                    do_oproj_unit()
            # final 4 t-blocks: 512-wide chunks from the now-free st
            # pool; copies alternate ACT/DVE (both idle by now)
            for tb in range(12, 16):
                ostage = osp.tile([128, E], BF16, tag="ostage",
                                  name="ostage")
                for ng in range(4):
                    op = stp.tile([128, 512], F32, tag="st", name="opf")
                    for jc in range(4):
                        nc.tensor.matmul(
                            op, attn[jc][:, tb * 128:(tb + 1) * 128],
                            wo_r[:, jc, ng * 512:(ng + 1) * 512],
                            start=(jc == 0), stop=(jc == 3),
                            skip_group_check=True)
                    sl = slice(ng * 512, (ng + 1) * 512)
                    if ng % 2 == 0:
                        nc.scalar.activation(
                            ostage[:, sl], op,
                            mybir.ActivationFunctionType.Copy)
                    else:
                        nc.vector.tensor_copy(ostage[:, sl], op)
                        h0 = (ng - 1) * 512
                        nc.sync.dma_start(
                            out=OUT[tb * 128:(tb + 1) * 128, h0:h0 + 1024],
                            in_=ostage[:, h0:h0 + 1024])

    nc.compile()
    return nc


def _prep_core_inputs(c, x, Wq, bq, Wk, bk, Wv, bv, Wo, xt_cache):
    g = c % 4
    b = c // 4
    if b not in xt_cache:
        xt_cache[b] = np.ascontiguousarray(
            x[b].T.reshape(EC, 128, S).transpose(1, 0, 2)).astype(BF)
    wq_s = Wq[:, 512 * g:512 * (g + 1)].reshape(E, 8, 64)
    wq_s = wq_s[:, HEAD_PERM, :].reshape(E, 512) * np.float32(0.125)
    wq = np.ascontiguousarray(
        wq_s.reshape(EC, 128, 4, 128).transpose(1, 2, 0, 3)).astype(BF)
    wk = np.ascontiguousarray(
        Wk[:, 128 * g:128 * (g + 1)].reshape(EC, 128, 128)
        .transpose(1, 0, 2)).astype(BF)
    wv = np.ascontiguousarray(
        Wv[:, 128 * g:128 * (g + 1)].reshape(EC, 128, 128)
        .transpose(1, 0, 2)).astype(BF)
    wo_s = Wo[512 * g:512 * (g + 1), :].reshape(8, 64, E)
    wo_s = wo_s[HEAD_PERM, :, :].reshape(512, E)
    wo = np.ascontiguousarray(
        wo_s.reshape(4, 128, E).transpose(1, 0, 2)).astype(BF)
    bias = np.zeros((128, 6), np.float32)
    bq_s = bq[512 * g:512 * (g + 1)].reshape(8, 64)[HEAD_PERM, :].reshape(512)
    bias[:, 0:4] = bq_s.reshape(4, 128).T * 0.125
    bias[:, 4] = bk[128 * g:128 * (g + 1)]
    biasv = np.tile(bv[128 * g:128 * (g + 1)][None, :],
                    (128, 1)).astype(np.float32)
    cmask = np.where(np.triu(np.ones((128, 128), bool)), 1.0,
                     0.0).astype(BF)
    return {"xt": xt_cache[b], "wq": wq, "wk": wk, "wv": wv, "wo": wo,
            "bias": bias, "biasv": biasv, "cmask": cmask}


def kernel(**inputs):
    from concourse.bass_utils import run_bass_kernel_spmd

    x = np.asarray(inputs["x"], np.float32)
    Wq = np.asarray(inputs["Wq"], np.float32)
    bq = np.asarray(inputs["bq"], np.float32)
    Wk = np.asarray(inputs["Wk"], np.float32)
    bk = np.asarray(inputs["bk"], np.float32)
    Wv = np.asarray(inputs["Wv"], np.float32)
    bv = np.asarray(inputs["bv"], np.float32)
    Wo = np.asarray(inputs["Wo"], np.float32)
    bo = np.asarray(inputs["bo"], np.float32)

    if "nc" not in _CACHE:
        _CACHE["nc"] = _build()
    nc = _CACHE["nc"]

    xt_cache = {}
    in_maps = [_prep_core_inputs(c, x, Wq, bq, Wk, bk, Wv, bv, Wo, xt_cache)
               for c in range(NCORE)]
    res = run_bass_kernel_spmd(nc, in_maps, list(range(NCORE)))
    parts = [np.asarray(res.results[c]["out"]).astype(np.float32)
             for c in range(NCORE)]
    out0 = parts[0] + parts[1] + parts[2] + parts[3] + bo
    out1 = parts[4] + parts[5] + parts[6] + parts[7] + bo
    return np.stack([out0, out1]).astype(np.float32)


# revision 88
# speedup vs baseline: 1.0003x; 1.0003x over previous
"""GQA kernel for Trainium2, 8 NeuronCores (all-bf16, fused pipeline).

Sharding: core c -> batch b = c//4, kv-head-group g = c%4.
Each core handles 1 batch, 2 KV heads (2g, 2g+1), 8 Q heads (8g..8g+7),
row-shard of W_o (rows 512g..512g+512). Host sums the 4 partial outputs
per batch and adds bo.

All matmul operands are bf16 (1 PE cycle/row at any moving size; fp32
PSUM accumulation), validated to ~4e-3 rel err vs the fp32 reference.

The kernel is ONE fused instruction stream: projections for t-group tg
are interleaved between attention head-blocks so PE always has
independent work while the Activation engine drains exponentials
(attention q-group qg only needs projections up to tg=2qg+1).

  1. QKV projections from x^T (E on partitions), weights DMA'd directly
     as bf16.  Q scaled by 1/8 on host.  V is produced directly in
     [t, d] layout by making x^T the stationary operand, with a ones
     column per key block so the attention matmul also yields the
     softmax denominator.
  2. Attention per (qg, qh), both kv chains kb-interleaved in key-block
     pairs: S^T = K^T.T Q^T per 128-wide key block (two blocks share one
     PSUM tile and one exp), causality via block skip + a post-exp 0/1
     triangular mask multiply on Pool (off the scores->exp chain), exp
     on ACT (no max subtraction: |scores| <= ~7), A V computed as
     out[q, 0:65] = at_block.T @ [V|1] (65-row moving operand).  The AV
     accumulator uses a single PSUM start (start=True zeroes the whole
     2KB zero region; later q-block regions rely on its pending-zero).
  3. Normalize rows by 1/l via DVE reciprocal + tensor_scalar multiply
     into a [q, d_kv0|d_kv1] tile, DMA-XBAR-transposed into attn[hh]
     as [d, q] (GPSIMD cannot touch PSUM; DVE drains all PSUM reads).
  4. O-projection partial = attn^T.T @ Wo_shard, drip-fed as 256-col
     units into the attention stream (paced per q-group), PSUM drained
     on DVE/ACT, streamed to DRAM in 1024-col halves.
"""

import numpy as np
import ml_dtypes

BF = ml_dtypes.bfloat16

E = 2048
S = 2048
B = 2
D = 64
NCORE = 8
TGW = 256          # t-column group width in projections
NTG = S // TGW     # 8
EC = E // 128      # 16 contraction chunks
NKB = S // 128     # 16 key blocks
NQG = S // 512     # 4 q column groups

_CACHE = {}
# tile jb holds q-heads (jb, jb+4): kv0 heads at base partition 0,
# kv1 heads at base partition 64, matching the K/V partition layout
HEAD_PERM = [0, 4, 1, 5, 2, 6, 3, 7]


def _build():
    import concourse.bass as bass
    import concourse.tile as tile
    from concourse import mybir, bacc

    F32 = mybir.dt.float32
    BF16 = mybir.dt.bfloat16
    Exp = mybir.ActivationFunctionType.Exp


    nc = bacc.Bacc("TRN2", target_bir_lowering=False, debug=False,
                   num_devices=NCORE)

    XT = nc.declare_dram_parameter("xt", [128, EC, S], BF16, isOutput=False)
    WQ = nc.declare_dram_parameter("wq", [128, 4, EC, 128], BF16,
                                   isOutput=False)
    WK = nc.declare_dram_parameter("wk", [128, EC, 128], BF16, isOutput=False)
    WV = nc.declare_dram_parameter("wv", [128, EC, 128], BF16, isOutput=False)
    WO = nc.declare_dram_parameter("wo", [128, 4, E], BF16, isOutput=False)
    BIAS = nc.declare_dram_parameter("bias", [128, 6], F32, isOutput=False)
    BIASV = nc.declare_dram_parameter("biasv", [128, 128], F32, isOutput=False)
    CM = nc.declare_dram_parameter("cmask", [128, 128], BF16, isOutput=False)
    # output streamed as bf16: halves the 16.8MB of output DMA (the
    # fp32->bf16 cast rides the existing PSUM-drain copies; host upcasts)
    OUT = nc.declare_dram_parameter("out", [S, E], BF16, isOutput=True)

    with tile.TileContext(nc) as tc:
        with tc.tile_pool(name="persist", bufs=1) as persist, \
             tc.tile_pool(name="xr", bufs=3) as xrp, \
             tc.tile_pool(name="at", bufs=18) as atp, \
             tc.tile_pool(name="nrm", bufs=10) as npool, \
             tc.tile_pool(name="rcp", bufs=8) as rpool, \
             tc.tile_pool(name="ostage", bufs=3) as osp, \
             tc.tile_pool(name="st", bufs=2, space="PSUM") as stp, \
             tc.tile_pool(name="av", bufs=2, space="PSUM") as avp, \
             tc.tile_pool(name="acc", bufs=2, space="PSUM") as accp:

            qt = [persist.tile([128, S], BF16, tag=f"qt{i}", name=f"qt{i}")
                  for i in range(4)]
            kt = persist.tile([128, S], BF16, tag="kt")
            # per key block kb: cols 0:64 = V_kv0, 64 = ones, 65:129 = V_kv1,
            # 129 = ones  (ones give the softmax denominator in the AV matmul)
            vt = persist.tile([128, NKB, 130], BF16, tag="vt")
            attn = [persist.tile([128, S], BF16, tag=f"attn{i}",
                                 name=f"attn{i}") for i in range(4)]
            wq_r = persist.tile([128, 4, EC, 128], BF16, tag="wq")
            wk_r = persist.tile([128, EC, 128], BF16, tag="wk")
            wv_r = persist.tile([128, EC, 128], BF16, tag="wv")
            wo_r = persist.tile([128, 4, E], BF16, tag="wo")
            # 0/1 upper-triangular mask, applied post-exp (keeps the Pool
            # engine off the scores->exp critical chain)
            cm = persist.tile([128, 128], BF16, tag="cm")
            bias_t = persist.tile([128, 6], F32, tag="bias")
            biasv_t = persist.tile([128, 128], F32, tag="biasv")

            # weight/bias DMAs, ordered so the first Q-proj chain can start
            # as early as possible (DMA engines are a serial resource)
            nc.sync.dma_start(out=wq_r[:, 0], in_=WQ[:, 0])
            xr0 = xrp.tile([128, EC, TGW], BF16, tag="xr", name="xr0")
            nc.sync.dma_start(out=xr0, in_=XT[:, :, 0:TGW])
            nc.sync.dma_start(out=wq_r[:, 1], in_=WQ[:, 1])
            nc.sync.dma_start(out=wq_r[:, 2], in_=WQ[:, 2])
            nc.sync.dma_start(out=wk_r, in_=WK[:, :, :])
            nc.sync.dma_start(out=wq_r[:, 3], in_=WQ[:, 3])
            nc.sync.dma_start(out=wv_r, in_=WV[:, :, :])
            nc.sync.dma_start(out=bias_t, in_=BIAS[:, :])
            nc.sync.dma_start(out=biasv_t, in_=BIASV[:, :])
            nc.sync.dma_start(out=cm, in_=CM[:, :])

            nc.gpsimd.memset(vt[:, :, 64:65], 1.0)
            nc.gpsimd.memset(vt[:, :, 129:130], 1.0)

            # ---- drip-fed filler units (projection chains + O-proj
            # chunks) interleaved into the attention kb loop: PE always has
            # independent ~0.5-1.7us work while ACT drains exponentials.
            xrs = {0: xr0}

            def xslice(tg, ec):
                return xrs[tg][:, ec, :]
            proj_q = []
            pending = []
            stage = {}

            def ensure_xr(tg):
                if tg in xrs or tg >= NTG:
                    return
                xr = xrp.tile([128, EC, TGW], BF16, tag="xr", name="xr")
                nc.sync.dma_start(out=xr,
                                  in_=XT[:, :, tg * TGW:(tg + 1) * TGW])
                xrs[tg] = xr

            def queue_proj(tg):
                proj_q.extend((tg, k) for k in range(7))

            def do_proj_unit():
                tg, k = proj_q.pop(0)
                if k == 0:
                    ensure_xr(tg + 1)
                cols = bass.ds(tg * TGW, TGW)
                if k < 5:
                    ps = accp.tile([128, TGW], F32, tag="acc", name="ps")
                    for ec in range(EC):
                        lhsT = wq_r[:, k, ec, :] if k < 4 else wk_r[:, ec, :]
                        nc.tensor.matmul(ps, lhsT, xslice(tg, ec),
                                         start=(ec == 0), stop=(ec == EC - 1))
                    if k < 4:
                        nc.vector.tensor_scalar_add(qt[k][:, cols], ps,
                                                    bias_t[:, k:k + 1])
                    else:
                        nc.vector.tensor_scalar_add(kt[:, cols], ps,
                                                    bias_t[:, 4:5])
                else:
                    # V directly in [t, d] layout: x^T chunk is stationary
                    h2 = k - 5
                    tb = 2 * tg + h2
                    psv = accp.tile([128, 128], F32, tag="acc", name="psv")
                    for ec in range(EC):
                        nc.tensor.matmul(
                            psv,
                            xslice(tg, ec)[:, h2 * 128:(h2 + 1) * 128],
                            wv_r[:, ec, :],
                            start=(ec == 0), stop=(ec == EC - 1))
                    nc.vector.tensor_add(vt[:, tb, 0:64], psv[:, 0:64],
                                         biasv_t[:, 0:64])
                    nc.vector.tensor_add(vt[:, tb, 65:129], psv[:, 64:128],
                                         biasv_t[:, 64:128])
                if tg == 1 and k == 6:
                    nc.sync.dma_start(out=wo_r, in_=WO[:, :, :])

            def queue_oproj(tb):
                pending.extend(('full', tb, ng) for ng in range(8))

            cur_qg = [0]
            flip = [0]

            def do_oproj_unit():
                kind, tb, ng = pending.pop(0)
                if ng == 0:
                    stage[tb] = osp.tile([128, E], BF16, tag="ostage",
                                         name="ostage")
                op = accp.tile([128, 256], F32, tag="acc", name="op")
                for jc in range(4):
                    nc.tensor.matmul(
                        op, attn[jc][:, tb * 128:(tb + 1) * 128],
                        wo_r[:, jc, ng * 256:(ng + 1) * 256],
                        start=(jc == 0), stop=(jc == 3),
                        skip_group_check=True)
                flip[0] ^= 1
                # ACT has slack until the attention stream deepens (qg>=2)
                if cur_qg[0] < 2 and flip[0]:
                    nc.scalar.activation(
                        stage[tb][:, ng * 256:(ng + 1) * 256], op,
                        mybir.ActivationFunctionType.Copy)
                else:
                    nc.vector.tensor_copy(
                        stage[tb][:, ng * 256:(ng + 1) * 256], op)
                if ng == 3 or ng == 7:
                    h0 = (ng - 3) * 256
                    nc.sync.dma_start(
                        out=OUT[tb * 128:(tb + 1) * 128, h0:h0 + 1024],
                        in_=stage[tb][:, h0:h0 + 1024])
                    if ng == 7:
                        stage.pop(tb)

            # pacing: drain the filler queues evenly over a qg's kb-substeps
            pace = {'n': 1, 'i': 0, 'tot': 0, 'done': 0}

            def fill_tick():
                pace['i'] += 1
                target = pace['tot'] * pace['i'] // pace['n']
                while pace['done'] < target and (proj_q or pending):
                    if proj_q:
                        do_proj_unit()
                    else:
                        do_oproj_unit()
                    pace['done'] += 1

            def attn_hh(qg, hh):
                q0 = qg * 512
                nkb = qg * 4 + 4
                # nrm2[qb] = normalized attn rows [q, d(kv0)|d(kv1)],
                # DMA-XBAR-transposed into attn[hh] as [d, q]
                nrm2 = [npool.tile([128, 128], BF16, tag="nrm",
                                   name=f"nrm{qb}") for qb in range(4)]
                # av[kv][:, qb, 0:65] accumulates [q, V|l] per head
                av = [avp.tile([128, 4, 128], F32, tag="av", name="av")
                      for _ in range(2)]
                pend = [[], []]

                def flush(kv, pair, at):
                    for kb, c0, off in pair:
                        for qb in range((c0 - q0) // 128, 4):
                            col = off + qb * 128 - (c0 - q0)
                            # start only on the tile's very first matmul:
                            # start=True zeroes the whole 2KB PSUM zero
                            # region, so later qb regions must rely on the
                            # pending-zero from that first start
                            nc.tensor.matmul(
                                av[kv][:, qb, 0:65],
                                at[:, col:col + 128],
                                vt[:, kb, kv * 65:kv * 65 + 65],
                                start=(kb == 0 and qb == 0),
                                stop=(kb == qg * 4 + qb),
                                skip_group_check=True)

                for p in range(nkb // 2):
                    for kv in range(2):
                        qoff = kv * 64
                        st = stp.tile([128, 1024], F32, tag="st", name="st")
                        pair = []
                        off = 0
                        for kb in (2 * p, 2 * p + 1):
                            c0 = max(kb * 128, q0)
                            w = q0 + 512 - c0
                            nc.tensor.matmul(
                                st[:, off:off + w],
                                kt[qoff:qoff + 64, kb * 128:(kb + 1) * 128],
                                qt[hh][qoff:qoff + 64, c0:c0 + w],
                                start=True, stop=True, skip_group_check=True)
                            pair.append((kb, c0, off))
                            off += w
                        at = atp.tile([128, 1024], BF16, tag="at", name="at")
                        nc.scalar.activation(at[:, :off], st[:, :off], Exp)
                        for kb, c0, koff in pair:
                            if kb * 128 >= q0:
                                nc.gpsimd.tensor_mul(at[:, koff:koff + 128],
                                                     at[:, koff:koff + 128],
                                                     cm)
                        pend[kv].append((pair, at))
                        if len(pend[kv]) > 7:
                            flush(kv, *pend[kv].pop(0))
                        fill_tick()
                for kv in range(2):
                    while pend[kv]:
                        flush(kv, *pend[kv].pop(0))
                    for qb in range(4):
                        rcp = rpool.tile([128, 1], F32, tag="rcp",
                                         name="rcp")
                        nc.vector.reciprocal(rcp, av[kv][:, qb, 64:65])
                        nc.vector.tensor_scalar_mul(
                            nrm2[qb][:, kv * 64:kv * 64 + 64],
                            av[kv][:, qb, 0:64], rcp)
                for qb in range(4):
                    qcol = q0 + qb * 128
                    nc.sync.dma_start_transpose(
                        out=attn[hh][:, qcol:qcol + 128], in_=nrm2[qb])

            # ---- fused schedule: projections feed attention just-in-time
            # (tg0/tg1 Q and K chains run up front; their V chains are only
            # needed by the first AV flush, so they drip in as early filler)
            proj_q.extend((tg, k) for tg in (0, 1) for k in range(5))
            for _ in range(10):
                do_proj_unit()
            proj_q.extend((tg, k) for tg in (0, 1) for k in (5, 6))
            for tg in range(2, NTG):
                queue_proj(tg)
            for qg in range(NQG):
                cur_qg[0] = qg
                for tb in range(max(qg - 1, 0) * 4, qg * 4):
                    queue_oproj(tb)
                nslots = 4 * (qg * 4 + 4)
                navail = len(proj_q) + len(pending)
                # early qgs: up to 1 unit per substep (surplus spills to the
                # next qg); qg3: spread the remaining units over all substeps
                tot = navail if qg == 3 else nslots
                if qg == 2:
                    tot = nslots - 32
                elif qg == 1:
                    tot = nslots - 16
                pace.update(n=nslots, i=0, done=0, tot=tot)
                for hh in range(4):
                    attn_hh(qg, hh)
            while proj_q or pending:
                if proj_q:
                    do_proj_unit()
                else:
                    do_oproj_unit()
            # final 4 t-blocks: 512-wide chunks from the now-free st
            # pool; copies alternate ACT/DVE (both idle by now)
            for tb in range(12, 16):
                ostage = osp.tile([128, E], BF16, tag="ostage",
                                  name="ostage")
                for ng in range(4):
                    op = stp.tile([128, 512], F32, tag="st", name="opf")
                    for jc in range(4):
                        nc.tensor.matmul(
                            op, attn[jc][:, tb * 128:(tb + 1) * 128],
                            wo_r[:, jc, ng * 512:(ng + 1) * 512],
                            start=(jc == 0), stop=(jc == 3),
                            skip_group_check=True)
                    sl = slice(ng * 512, (ng + 1) * 512)
                    if ng % 2 == 0:
                        nc.scalar.activation(
                            ostage[:, sl], op,
                            mybir.ActivationFunctionType.Copy)
                    else:
                        nc.vector.tensor_copy(ostage[:, sl], op)
                        h0 = (ng - 1) * 512
                        nc.sync.dma_start(
                            out=OUT[tb * 128:(tb + 1) * 128, h0:h0 + 1024],
                            in_=ostage[:, h0:h0 + 1024])

    nc.compile()
    return nc


def _prep_core_inputs(c, x, Wq, bq, Wk, bk, Wv, bv, Wo, xt_cache):
    g = c % 4
    b = c // 4
    if b not in xt_cache:
        xt_cache[b] = np.ascontiguousarray(
            x[b].T.reshape(EC, 128, S).transpose(1, 0, 2)).astype(BF)
    wq_s = Wq[:, 512 * g:512 * (g + 1)].reshape(E, 8, 64)
    wq_s = wq_s[:, HEAD_PERM, :].reshape(E, 512) * np.float32(0.125)
    wq = np.ascontiguousarray(
        wq_s.reshape(EC, 128, 4, 128).transpose(1, 2, 0, 3)).astype(BF)
    wk = np.ascontiguousarray(
        Wk[:, 128 * g:128 * (g + 1)].reshape(EC, 128, 128)
        .transpose(1, 0, 2)).astype(BF)
    wv = np.ascontiguousarray(
        Wv[:, 128 * g:128 * (g + 1)].reshape(EC, 128, 128)
        .transpose(1, 0, 2)).astype(BF)
    wo_s = Wo[512 * g:512 * (g + 1), :].reshape(8, 64, E)
    wo_s = wo_s[HEAD_PERM, :, :].reshape(512, E)
    wo = np.ascontiguousarray(
        wo_s.reshape(4, 128, E).transpose(1, 0, 2)).astype(BF)
    bias = np.zeros((128, 6), np.float32)
    bq_s = bq[512 * g:512 * (g + 1)].reshape(8, 64)[HEAD_PERM, :].reshape(512)
    bias[:, 0:4] = bq_s.reshape(4, 128).T * 0.125
    bias[:, 4] = bk[128 * g:128 * (g + 1)]
    biasv = np.tile(bv[128 * g:128 * (g + 1)][None, :],
                    (128, 1)).astype(np.float32)
    cmask = np.where(np.triu(np.ones((128, 128), bool)), 1.0,
                     0.0).astype(BF)
    return {"xt": xt_cache[b], "wq": wq, "wk": wk, "wv": wv, "wo": wo,
            "bias": bias, "biasv": biasv, "cmask": cmask}


def kernel(**inputs):
    from concourse.bass_utils import run_bass_kernel_spmd

    x = np.asarray(inputs["x"], np.float32)
    Wq = np.asarray(inputs["Wq"], np.float32)
    bq = np.asarray(inputs["bq"], np.float32)
    Wk = np.asarray(inputs["Wk"], np.float32)
    bk = np.asarray(inputs["bk"], np.float32)
    Wv = np.asarray(inputs["Wv"], np.float32)
    bv = np.asarray(inputs["bv"], np.float32)
    Wo = np.asarray(inputs["Wo"], np.float32)
    bo = np.asarray(inputs["bo"], np.float32)

    if "nc" not in _CACHE:
        _CACHE["nc"] = _build()
    nc = _CACHE["nc"]

    xt_cache = {}
    in_maps = [_prep_core_inputs(c, x, Wq, bq, Wk, bk, Wv, bv, Wo, xt_cache)
               for c in range(NCORE)]
    res = run_bass_kernel_spmd(nc, in_maps, list(range(NCORE)))
    parts = [np.asarray(res.results[c]["out"]).astype(np.float32)
             for c in range(NCORE)]
    out0 = parts[0] + parts[1] + parts[2] + parts[3] + bo
    out1 = parts[4] + parts[5] + parts[6] + parts[7] + bo
    return np.stack([out0, out1]).astype(np.float32)


# revision 93
# speedup vs baseline: 1.0006x; 1.0003x over previous
"""GQA kernel for Trainium2, 8 NeuronCores (all-bf16, fused pipeline).

Sharding: core c -> batch b = c//4, kv-head-group g = c%4.
Each core handles 1 batch, 2 KV heads (2g, 2g+1), 8 Q heads (8g..8g+7),
row-shard of W_o (rows 512g..512g+512). Host sums the 4 partial outputs
per batch and adds bo.

All matmul operands are bf16 (1 PE cycle/row at any moving size; fp32
PSUM accumulation), validated to ~4e-3 rel err vs the fp32 reference.

The kernel is ONE fused instruction stream: projections for t-group tg
are interleaved between attention head-blocks so PE always has
independent work while the Activation engine drains exponentials
(attention q-group qg only needs projections up to tg=2qg+1).

  1. QKV projections from x^T (E on partitions), weights DMA'd directly
     as bf16.  Q scaled by 1/8 on host.  V is produced directly in
     [t, d] layout by making x^T the stationary operand, with a ones
     column per key block so the attention matmul also yields the
     softmax denominator.
  2. Attention per (qg, qh), both kv chains kb-interleaved in key-block
     pairs: S^T = K^T.T Q^T per 128-wide key block (two blocks share one
     PSUM tile and one exp), causality via block skip + a post-exp 0/1
     triangular mask multiply on Pool (off the scores->exp chain), exp
     on ACT (no max subtraction: |scores| <= ~7), A V computed as
     out[q, 0:65] = at_block.T @ [V|1] (65-row moving operand).  The AV
     accumulator uses a single PSUM start (start=True zeroes the whole
     2KB zero region; later q-block regions rely on its pending-zero).
  3. Normalize rows by 1/l via DVE reciprocal + tensor_scalar multiply
     into a [q, d_kv0|d_kv1] tile, DMA-XBAR-transposed into attn[hh]
     as [d, q] (GPSIMD cannot touch PSUM; DVE drains all PSUM reads).
  4. O-projection partial = attn^T.T @ Wo_shard, drip-fed as 256-col
     units into the attention stream (paced per q-group), PSUM drained
     on DVE/ACT, streamed to DRAM in 1024-col halves.
"""

import numpy as np
import ml_dtypes

BF = ml_dtypes.bfloat16

E = 2048
S = 2048
B = 2
D = 64
NCORE = 8
TGW = 256          # t-column group width in projections
NTG = S // TGW     # 8
EC = E // 128      # 16 contraction chunks
NKB = S // 128     # 16 key blocks
NQG = S // 512     # 4 q column groups

_CACHE = {}
# tile jb holds q-heads (jb, jb+4): kv0 heads at base partition 0,
# kv1 heads at base partition 64, matching the K/V partition layout
HEAD_PERM = [0, 4, 1, 5, 2, 6, 3, 7]


def _build():
    import concourse.bass as bass
    import concourse.tile as tile
    from concourse import mybir, bacc

    F32 = mybir.dt.float32
    BF16 = mybir.dt.bfloat16
    Exp = mybir.ActivationFunctionType.Exp


    nc = bacc.Bacc("TRN2", target_bir_lowering=False, debug=False,
                   num_devices=NCORE)

    XT = nc.declare_dram_parameter("xt", [128, EC, S], BF16, isOutput=False)
    WQ = nc.declare_dram_parameter("wq", [128, 4, EC, 128], BF16,
                                   isOutput=False)
    WK = nc.declare_dram_parameter("wk", [128, EC, 128], BF16, isOutput=False)
    WV = nc.declare_dram_parameter("wv", [128, EC, 128], BF16, isOutput=False)
    WO = nc.declare_dram_parameter("wo", [128, 4, E], BF16, isOutput=False)
    BIAS = nc.declare_dram_parameter("bias", [128, 6], F32, isOutput=False)
    BIASV = nc.declare_dram_parameter("biasv", [128, 128], F32, isOutput=False)
    CM = nc.declare_dram_parameter("cmask", [128, 128], BF16, isOutput=False)
    # output streamed as bf16: halves the 16.8MB of output DMA (the
    # fp32->bf16 cast rides the existing PSUM-drain copies; host upcasts)
    OUT = nc.declare_dram_parameter("out", [S, E], BF16, isOutput=True)

    with tile.TileContext(nc) as tc:
        with tc.tile_pool(name="persist", bufs=1) as persist, \
             tc.tile_pool(name="xr", bufs=3) as xrp, \
             tc.tile_pool(name="at", bufs=18) as atp, \
             tc.tile_pool(name="nrm", bufs=10) as npool, \
             tc.tile_pool(name="rcp", bufs=8) as rpool, \
             tc.tile_pool(name="ostage", bufs=4) as osp, \
             tc.tile_pool(name="st", bufs=2, space="PSUM") as stp, \
             tc.tile_pool(name="av", bufs=2, space="PSUM") as avp, \
             tc.tile_pool(name="acc", bufs=2, space="PSUM") as accp:

            qt = [persist.tile([128, S], BF16, tag=f"qt{i}", name=f"qt{i}")
                  for i in range(4)]
            kt = persist.tile([128, S], BF16, tag="kt")
            # per key block kb: cols 0:64 = V_kv0, 64 = ones, 65:129 = V_kv1,
            # 129 = ones  (ones give the softmax denominator in the AV matmul)
            vt = persist.tile([128, NKB, 130], BF16, tag="vt")
            attn = [persist.tile([128, S], BF16, tag=f"attn{i}",
                                 name=f"attn{i}") for i in range(4)]
            wq_r = persist.tile([128, 4, EC, 128], BF16, tag="wq")
            wk_r = persist.tile([128, EC, 128], BF16, tag="wk")
            wv_r = persist.tile([128, EC, 128], BF16, tag="wv")
            wo_r = persist.tile([128, 4, E], BF16, tag="wo")
            # 0/1 upper-triangular mask, applied post-exp (keeps the Pool
            # engine off the scores->exp critical chain)
            cm = persist.tile([128, 128], BF16, tag="cm")
            bias_t = persist.tile([128, 6], F32, tag="bias")
            biasv_t = persist.tile([128, 128], F32, tag="biasv")

            # weight/bias DMAs, ordered so the first Q-proj chain can start
            # as early as possible (DMA engines are a serial resource)
            nc.sync.dma_start(out=wq_r[:, 0], in_=WQ[:, 0])
            xr0 = xrp.tile([128, EC, TGW], BF16, tag="xr", name="xr0")
            nc.sync.dma_start(out=xr0, in_=XT[:, :, 0:TGW])
            nc.sync.dma_start(out=wq_r[:, 1], in_=WQ[:, 1])
            nc.sync.dma_start(out=wq_r[:, 2], in_=WQ[:, 2])
            nc.sync.dma_start(out=wk_r, in_=WK[:, :, :])
            nc.sync.dma_start(out=wq_r[:, 3], in_=WQ[:, 3])
            nc.sync.dma_start(out=wv_r, in_=WV[:, :, :])
            nc.sync.dma_start(out=bias_t, in_=BIAS[:, :])
            nc.sync.dma_start(out=biasv_t, in_=BIASV[:, :])
            nc.sync.dma_start(out=cm, in_=CM[:, :])

            nc.gpsimd.memset(vt[:, :, 64:65], 1.0)
            nc.gpsimd.memset(vt[:, :, 129:130], 1.0)

            # ---- drip-fed filler units (projection chains + O-proj
            # chunks) interleaved into the attention kb loop: PE always has
            # independent ~0.5-1.7us work while ACT drains exponentials.
            xrs = {0: xr0}

            def xslice(tg, ec):
                return xrs[tg][:, ec, :]
            proj_q = []
            pending = []
            stage = {}

            def ensure_xr(tg):
                if tg in xrs or tg >= NTG:
                    return
                xr = xrp.tile([128, EC, TGW], BF16, tag="xr", name="xr")
                nc.sync.dma_start(out=xr,
                                  in_=XT[:, :, tg * TGW:(tg + 1) * TGW])
                xrs[tg] = xr

            def queue_proj(tg):
                proj_q.extend((tg, k) for k in range(7))

            def do_proj_unit():
                tg, k = proj_q.pop(0)
                if k == 0:
                    ensure_xr(tg + 1)
                cols = bass.ds(tg * TGW, TGW)
                if k < 5:
                    ps = accp.tile([128, TGW], F32, tag="acc", name="ps")
                    for ec in range(EC):
                        lhsT = wq_r[:, k, ec, :] if k < 4 else wk_r[:, ec, :]
                        nc.tensor.matmul(ps, lhsT, xslice(tg, ec),
                                         start=(ec == 0), stop=(ec == EC - 1))
                    if k < 4:
                        nc.vector.tensor_scalar_add(qt[k][:, cols], ps,
                                                    bias_t[:, k:k + 1])
                    else:
                        nc.vector.tensor_scalar_add(kt[:, cols], ps,
                                                    bias_t[:, 4:5])
                else:
                    # V directly in [t, d] layout: x^T chunk is stationary
                    h2 = k - 5
                    tb = 2 * tg + h2
                    psv = accp.tile([128, 128], F32, tag="acc", name="psv")
                    for ec in range(EC):
                        nc.tensor.matmul(
                            psv,
                            xslice(tg, ec)[:, h2 * 128:(h2 + 1) * 128],
                            wv_r[:, ec, :],
                            start=(ec == 0), stop=(ec == EC - 1))
                    nc.vector.tensor_add(vt[:, tb, 0:64], psv[:, 0:64],
                                         biasv_t[:, 0:64])
                    nc.vector.tensor_add(vt[:, tb, 65:129], psv[:, 64:128],
                                         biasv_t[:, 64:128])
                if tg == 1 and k == 6:
                    nc.sync.dma_start(out=wo_r, in_=WO[:, :, :])

            def queue_oproj(tb):
                pending.extend(('full', tb, ng) for ng in range(8))

            cur_qg = [0]
            flip = [0]

            def do_oproj_unit():
                kind, tb, ng = pending.pop(0)
                if ng == 0:
                    stage[tb] = osp.tile([128, E], BF16, tag="ostage",
                                         name="ostage")
                op = accp.tile([128, 256], F32, tag="acc", name="op")
                for jc in range(4):
                    nc.tensor.matmul(
                        op, attn[jc][:, tb * 128:(tb + 1) * 128],
                        wo_r[:, jc, ng * 256:(ng + 1) * 256],
                        start=(jc == 0), stop=(jc == 3),
                        skip_group_check=True)
                flip[0] ^= 1
                # ACT has slack until the attention stream deepens (qg>=2)
                if cur_qg[0] < 2 and flip[0]:
                    nc.scalar.activation(
                        stage[tb][:, ng * 256:(ng + 1) * 256], op,
                        mybir.ActivationFunctionType.Copy)
                else:
                    nc.vector.tensor_copy(
                        stage[tb][:, ng * 256:(ng + 1) * 256], op)
                if ng == 3 or ng == 7:
                    h0 = (ng - 3) * 256
                    nc.sync.dma_start(
                        out=OUT[tb * 128:(tb + 1) * 128, h0:h0 + 1024],
                        in_=stage[tb][:, h0:h0 + 1024])
                    if ng == 7:
                        stage.pop(tb)

            # pacing: drain the filler queues evenly over a qg's kb-substeps
            pace = {'n': 1, 'i': 0, 'tot': 0, 'done': 0}

            def fill_tick():
                pace['i'] += 1
                target = pace['tot'] * pace['i'] // pace['n']
                while pace['done'] < target and (proj_q or pending):
                    if proj_q:
                        do_proj_unit()
                    else:
                        do_oproj_unit()
                    pace['done'] += 1

            def attn_hh(qg, hh):
                q0 = qg * 512
                nkb = qg * 4 + 4
                # nrm2[qb] = normalized attn rows [q, d(kv0)|d(kv1)],
                # DMA-XBAR-transposed into attn[hh] as [d, q]
                nrm2 = [npool.tile([128, 128], BF16, tag="nrm",
                                   name=f"nrm{qb}") for qb in range(4)]
                # av[kv][:, qb, 0:65] accumulates [q, V|l] per head
                av = [avp.tile([128, 4, 128], F32, tag="av", name="av")
                      for _ in range(2)]
                pend = [[], []]

                def flush(kv, pair, at):
                    for kb, c0, off in pair:
                        for qb in range((c0 - q0) // 128, 4):
                            col = off + qb * 128 - (c0 - q0)
                            # start only on the tile's very first matmul:
                            # start=True zeroes the whole 2KB PSUM zero
                            # region, so later qb regions must rely on the
                            # pending-zero from that first start
                            nc.tensor.matmul(
                                av[kv][:, qb, 0:65],
                                at[:, col:col + 128],
                                vt[:, kb, kv * 65:kv * 65 + 65],
                                start=(kb == 0 and qb == 0),
                                stop=(kb == qg * 4 + qb),
                                skip_group_check=True)

                for p in range(nkb // 2):
                    for kv in range(2):
                        qoff = kv * 64
                        st = stp.tile([128, 1024], F32, tag="st", name="st")
                        pair = []
                        off = 0
                        for kb in (2 * p, 2 * p + 1):
                            c0 = max(kb * 128, q0)
                            w = q0 + 512 - c0
                            nc.tensor.matmul(
                                st[:, off:off + w],
                                kt[qoff:qoff + 64, kb * 128:(kb + 1) * 128],
                                qt[hh][qoff:qoff + 64, c0:c0 + w],
                                start=True, stop=True, skip_group_check=True)
                            pair.append((kb, c0, off))
                            off += w
                        at = atp.tile([128, 1024], BF16, tag="at", name="at")
                        nc.scalar.activation(at[:, :off], st[:, :off], Exp)
                        for kb, c0, koff in pair:
                            if kb * 128 >= q0:
                                nc.gpsimd.tensor_mul(at[:, koff:koff + 128],
                                                     at[:, koff:koff + 128],
                                                     cm)
                        pend[kv].append((pair, at))
                        if len(pend[kv]) > 7:
                            flush(kv, *pend[kv].pop(0))
                        fill_tick()
                for kv in range(2):
                    while pend[kv]:
                        flush(kv, *pend[kv].pop(0))
                    for qb in range(4):
                        rcp = rpool.tile([128, 1], F32, tag="rcp",
                                         name="rcp")
                        nc.vector.reciprocal(rcp, av[kv][:, qb, 64:65])
                        nc.vector.tensor_scalar_mul(
                            nrm2[qb][:, kv * 64:kv * 64 + 64],
                            av[kv][:, qb, 0:64], rcp)
                for qb in range(4):
                    qcol = q0 + qb * 128
                    nc.sync.dma_start_transpose(
                        out=attn[hh][:, qcol:qcol + 128], in_=nrm2[qb])

            # ---- fused schedule: projections feed attention just-in-time
            # (tg0/tg1 Q and K chains run up front; their V chains are only
            # needed by the first AV flush, so they drip in as early filler)
            proj_q.extend((tg, k) for tg in (0, 1) for k in range(5))
            for _ in range(10):
                do_proj_unit()
            proj_q.extend((tg, k) for tg in (0, 1) for k in (5, 6))
            for tg in range(2, NTG):
                queue_proj(tg)
            for qg in range(NQG):
                cur_qg[0] = qg
                for tb in range(max(qg - 1, 0) * 4, qg * 4):
                    queue_oproj(tb)
                nslots = 4 * (qg * 4 + 4)
                navail = len(proj_q) + len(pending)
                # early qgs: up to 1 unit per substep (surplus spills to the
                # next qg); qg3: spread the remaining units over all substeps
                tot = navail if qg == 3 else nslots
                if qg == 2:
                    tot = nslots - 32
                elif qg == 1:
                    tot = nslots - 16
                pace.update(n=nslots, i=0, done=0, tot=tot)
                for hh in range(4):
                    attn_hh(qg, hh)
            while proj_q or pending:
                if proj_q:
                    do_proj_unit()
                else:
                    do_oproj_unit()
            # final 4 t-blocks: 512-wide chunks from the now-free st
            # pool; copies alternate ACT/DVE (both idle by now)
            for tb in range(12, 16):
                ostage = osp.tile([128, E], BF16, tag="ostage",
                                  name="ostage")
                for ng in range(4):
                    op = stp.tile([128, 512], F32, tag="st", name="opf")
                    for jc in range(4):
                        nc.tensor.matmul(
                            op, attn[jc][:, tb * 128:(tb + 1) * 128],
                            wo_r[:, jc, ng * 512:(ng + 1) * 512],
                            start=(jc == 0), stop=(jc == 3),
                            skip_group_check=True)
                    sl = slice(ng * 512, (ng + 1) * 512)
                    if ng % 2 == 0:
                        nc.scalar.activation(
                            ostage[:, sl], op,
                            mybir.ActivationFunctionType.Copy)
                    else:
                        nc.vector.tensor_copy(ostage[:, sl], op)
                        h0 = (ng - 1) * 512
                        nc.sync.dma_start(
                            out=OUT[tb * 128:(tb + 1) * 128, h0:h0 + 1024],
                            in_=ostage[:, h0:h0 + 1024])

    nc.compile()
    return nc


def _prep_core_inputs(c, x, Wq, bq, Wk, bk, Wv, bv, Wo, xt_cache):
    g = c % 4
    b = c // 4
    if b not in xt_cache:
        xt_cache[b] = np.ascontiguousarray(
            x[b].T.reshape(EC, 128, S).transpose(1, 0, 2)).astype(BF)
    wq_s = Wq[:, 512 * g:512 * (g + 1)].reshape(E, 8, 64)
    wq_s = wq_s[:, HEAD_PERM, :].reshape(E, 512) * np.float32(0.125)
    wq = np.ascontiguousarray(
        wq_s.reshape(EC, 128, 4, 128).transpose(1, 2, 0, 3)).astype(BF)
    wk = np.ascontiguousarray(
        Wk[:, 128 * g:128 * (g + 1)].reshape(EC, 128, 128)
        .transpose(1, 0, 2)).astype(BF)
    wv = np.ascontiguousarray(
        Wv[:, 128 * g:128 * (g + 1)].reshape(EC, 128, 128)
        .transpose(1, 0, 2)).astype(BF)
    wo_s = Wo[512 * g:512 * (g + 1), :].reshape(8, 64, E)
    wo_s = wo_s[HEAD_PERM, :, :].reshape(512, E)
    wo = np.ascontiguousarray(
        wo_s.reshape(4, 128, E).transpose(1, 0, 2)).astype(BF)
    bias = np.zeros((128, 6), np.float32)
    bq_s = bq[512 * g:512 * (g + 1)].reshape(8, 64)[HEAD_PERM, :].reshape(512)
    bias[:, 0:4] = bq_s.reshape(4, 128).T * 0.125
    bias[:, 4] = bk[128 * g:128 * (g + 1)]
    biasv = np.tile(bv[128 * g:128 * (g + 1)][None, :],
                    (128, 1)).astype(np.float32)
    cmask = np.where(np.triu(np.ones((128, 128), bool)), 1.0,
                     0.0).astype(BF)
    return {"xt": xt_cache[b], "wq": wq, "wk": wk, "wv": wv, "wo": wo,
            "bias": bias, "biasv": biasv, "cmask": cmask}


def kernel(**inputs):
    from concourse.bass_utils import run_bass_kernel_spmd

    x = np.asarray(inputs["x"], np.float32)
    Wq = np.asarray(inputs["Wq"], np.float32)
    bq = np.asarray(inputs["bq"], np.float32)
    Wk = np.asarray(inputs["Wk"], np.float32)
    bk = np.asarray(inputs["bk"], np.float32)
    Wv = np.asarray(inputs["Wv"], np.float32)
    bv = np.asarray(inputs["bv"], np.float32)
    Wo = np.asarray(inputs["Wo"], np.float32)
    bo = np.asarray(inputs["bo"], np.float32)

    if "nc" not in _CACHE:
        _CACHE["nc"] = _build()
    nc = _CACHE["nc"]

    xt_cache = {}
    in_maps = [_prep_core_inputs(c, x, Wq, bq, Wk, bk, Wv, bv, Wo, xt_cache)
               for c in range(NCORE)]
    res = run_bass_kernel_spmd(nc, in_maps, list(range(NCORE)))
    parts = [np.asarray(res.results[c]["out"]).astype(np.float32)
             for c in range(NCORE)]
    out0 = parts[0] + parts[1] + parts[2] + parts[3] + bo
    out1 = parts[4] + parts[5] + parts[6] + parts[7] + bo
    return np.stack([out0, out1]).astype(np.float32)
